# revision 11
# baseline (speedup 1.0000x reference)
"""DeepFM (embedding_lookup) Trainium2 Bass kernel, v2: multi-engine histogram.

Math: with idx[b,f,c] = sparse_feat[:, f*(C+1)+c] (c in [0,C)), all lookups of
field f hit the C-row table emb_tab[f].  Let count[b, r] with r=(v,f) be the
exact histogram of idx over the 64 bin values v.  Then with the host-fused
table T'[r] = [emb_row | -0.5*|row|^2 | lin_emb*lin_W] (bf16, (v,f)-row-major):

  y_t[0:64]  = s,   y_t[64] = -0.5*sqsum,   y_t[65] = lin (+ dense@W_d + b)
  out = sigmoid(0.5*|s|^2 + y_t[64] + y_t[65])

Per core (256 batch rows), the 64 one-hot bins are split across engines:
  - class A: DVE tensor_scalar compare (4x mode), q-sum levels 1+2 on
    SWDGE accumulate-DMAs
  - class B: DVE compare of half + scalar_tensor_tensor fold of the other
    half (fuses level 1), level 2 on accumulate-DMA
  - class C: GPSIMD 4-op compare chain (folds levels 1+2 in the chain)
  - class D: ACT one-hot via Relu(1-(idx-v)^2) (exact for int idx),
    levels 1+2 on accumulate-DMAs
  - remaining c-sum tree levels: DVE bf16 tensor_tensor at 2x, merged
    across each group of 8 bins
  - count (bf16, exact ints) -> PE transposes -> bf16 K=1664 matmul
    against T' -> tiny PE epilogue + ACT sigmoid
"""

import numpy as np
import ml_dtypes
from contextlib import ExitStack

import concourse.bass as bass
import concourse.mybir as mybir
import concourse.tile as tile
from concourse import bacc
from concourse.bass_utils import run_bass_kernel_spmd
from concourse.masks import make_identity

B, F, C, E, D = 2048, 26, 64, 64, 13
NCORES = 8
BC = B // NCORES          # 256 batch rows per core
NBT = BC // 128           # 2 batch tiles of 128
R = F * C                 # 1664 table rows, (v,f)-major: r = v*F + f
NK = R // 128             # 13 K-chunks of 128
VG = 8                    # bins per tail group
NG = C // VG              # 8 groups
QTR = C // 4              # 16 c-values per quarter block
FP32 = mybir.dt.float32
BF16 = mybir.dt.bfloat16

# staged column order: j = q*416 + cc*26 + f  ->  original col f*(C+1)+q*16+cc
_COLS = np.array(
    [
        f * (C + 1) + q * QTR + cc
        for q in range(4)
        for cc in range(QTR)
        for f in range(F)
    ]
)

# per-group class pattern for the 8 bins (v-slots) of each tail group:
#   'A' full compare + DMA l1+l2 on DVE, 'B' DVE fold (STT) + DMA l2,
#   'C' like A but compare on gpsimd, 'D' like A but compare on ACT.
# A-runs and B-runs are kept contiguous so their accum-DMAs merge.
# totals: A=18, B=30, C=8, D=8
_PATTERNS = [list("CDAABBBB") for _ in range(6)] + [
    list("CDAAABBB") for _ in range(2)
]

_BUILT = {}


def _emit(ctx, tc, idx_d, dense_d, tab_d, y_d):
    nc = tc.nc
    ts = bass.ts
    E_ = mybir.AluOpType
    AF = mybir.ActivationFunctionType

    consts = ctx.enter_context(tc.tile_pool(name="consts", bufs=1))
    work = ctx.enter_context(tc.tile_pool(name="work", bufs=1))
    small = ctx.enter_context(tc.tile_pool(name="small", bufs=2))
    ohpool = ctx.enter_context(tc.tile_pool(name="ohpool", bufs=3))
    actpool = ctx.enter_context(tc.tile_pool(name="actpool", bufs=2))
    psum = ctx.enter_context(tc.tile_pool(name="psum", bufs=2, space="PSUM"))
    psum1 = ctx.enter_context(tc.tile_pool(name="psum1", bufs=1, space="PSUM"))

    # ---- input DMAs: idx in 4 q-chunks so compares start early ----
    idx_sb = work.tile([128, NBT, 4, QTR * F], BF16)
    idx_src = idx_d.ap().rearrange("(bt p) (q x) -> p bt q x", p=128, q=4)
    for q in range(4):
        nc.sync.dma_start(out=idx_sb[:, :, q, :], in_=idx_src[:, :, q, :])

    # host-fused table T' [p, kt, 66] bf16 and dense/bias chunk [14, bt, 128]
    tabT = work.tile([128, NK, 66], BF16)
    nc.scalar.dma_start(
        out=tabT, in_=tab_d.ap()[0:R, :].rearrange("(kt p) c -> p kt c", p=128)
    )
    densebias_t = work.tile([14, NBT, 128], BF16)
    nc.scalar.dma_start(out=densebias_t, in_=dense_d.ap())
    wdense = work.tile([14, 66], BF16)
    nc.scalar.dma_start(out=wdense, in_=tab_d.ap()[R : R + 14, :])

    # ---- constants ----
    ident = consts.tile([128, 128], BF16)
    make_identity(nc, ident)
    half_col = consts.tile([64, 1], FP32)
    nc.gpsimd.memset(half_col, 0.5)
    ones2 = consts.tile([2, 1], FP32)
    nc.gpsimd.memset(ones2, 1.0)
    one_ap = consts.tile([128, 1], FP32)
    nc.gpsimd.memset(one_ap, 1.0)
    negone_ap = consts.tile([128, 1], FP32)
    nc.gpsimd.memset(negone_ap, -1.0)
    # per-ACT-bin bias values -v, one column each
    act_bins = [
        (g, j) for g in range(NG) for j in range(VG) if _PATTERNS[g][j] == "D"
    ]
    dbias = consts.tile([128, len(act_bins)], FP32)
    for i, (g, j) in enumerate(act_bins):
        nc.gpsimd.memset(dbias[:, i : i + 1], -float(g * VG + j))

    count = work.tile([128, NBT, C, F], BF16)  # count[p, bt, v, f]
    ct_all = work.tile([128, NK, NBT, 128], BF16)

    idx_full = idx_sb.rearrange("p bt q x -> p bt (q x)")

    # ---- histogram: per group of 8 bins ----
    def compares(g, ohG):
        """Emit one group's 8 bin one-hots into ohG [p, v8, bt, q4, 416]."""
        # runs of consecutive same-class slots whose accum-DMAs merge
        runs = []
        for j in range(VG):
            cls = _PATTERNS[g][j]
            if runs and runs[-1][0] == cls:
                runs[-1][2] = j + 1
            else:
                runs.append([cls, j, j + 1])
        for j in range(VG):
            v = float(g * VG + j)
            cls = _PATTERNS[g][j]
            slot = ohG[:, j, :, :, :]
            if cls == "A":
                nc.vector.tensor_scalar(
                    out=slot.rearrange("p bt q x -> p bt (q x)"),
                    in0=idx_full,
                    scalar1=v,
                    scalar2=None,
                    op0=E_.is_equal,
                )
            elif cls == "C":
                nc.gpsimd.tensor_scalar(
                    out=slot.rearrange("p bt q x -> p bt (q x)"),
                    in0=idx_full,
                    scalar1=v,
                    scalar2=None,
                    op0=E_.is_equal,
                )
            elif cls == "D":
                i = act_bins.index((g, j))
                tmp = actpool.tile([128, NBT, 4, QTR * F], BF16, tag="acttmp")
                nc.scalar.activation(
                    out=tmp.rearrange("p bt q x -> p bt (q x)"),
                    in_=idx_full,
                    func=AF.Square,
                    bias=dbias[:, i : i + 1],
                )
                nc.scalar.activation(
                    out=slot.rearrange("p bt q x -> p bt (q x)"),
                    in_=tmp.rearrange("p bt q x -> p bt (q x)"),
                    func=AF.Relu,
                    bias=one_ap,
                    scale=negone_ap,
                )
            else:  # 'B': compare q23, fold q01 via STT (fuses level 1)
                nc.vector.tensor_scalar(
                    out=slot[:, :, 0:2, :],
                    in0=idx_sb[:, :, 2:4, :],
                    scalar1=v,
                    scalar2=None,
                    op0=E_.is_equal,
                )
                nc.vector.scalar_tensor_tensor(
                    out=slot[:, :, 0:2, :],
                    in0=idx_sb[:, :, 0:2, :],
                    scalar=v,
                    in1=slot[:, :, 0:2, :],
                    op0=E_.is_equal,
                    op1=E_.add,
                )
        # accum-DMAs, one launch per contiguous same-class run
        for cls, j0, j1 in runs:
            blk = ohG[:, j0:j1, :, :, :]
            if cls in ("A", "C", "D"):
                nc.gpsimd.dma_start(
                    out=blk[:, :, :, 0:2, :],
                    in_=blk[:, :, :, 2:4, :],
                    accum_op=E_.add,
                )
            nc.gpsimd.dma_start(
                out=blk[:, :, :, 0, :], in_=blk[:, :, :, 1, :], accum_op=E_.add
            )

    def tail(g, ohG):
        """c-sum levels 3..6 for group g: ohG[:, :, :, 0, :] -> count."""
        # view [p, v, bt, c16, f]; halve c until 1
        blk = ohG[:, :, :, 0, :].rearrange("p v bt (c f) -> p v bt c f", f=F)
        h = QTR // 2
        while h >= 2:
            a = blk[:, :, :, 0:h, :].rearrange("p v bt c f -> p v bt (c f)")
            b = blk[:, :, :, h : 2 * h, :].rearrange("p v bt c f -> p v bt (c f)")
            nc.vector.tensor_tensor(out=a, in0=a, in1=b, op=E_.add)
            h //= 2
        nc.vector.tensor_tensor(
            out=count[:, :, ts(g, VG), :].rearrange("p bt v f -> p v bt f"),
            in0=blk[:, :, :, 0, :],
            in1=blk[:, :, :, 1, :],
            op=E_.add,
        )

    # transposes unlocked after each group's tail: kt chunks fully covered
    done_r = [min((g + 1) * VG * F, R) for g in range(NG)]
    kt_ready = []
    prev = 0
    for g in range(NG):
        hi = done_r[g] // 128
        kt_ready.append(list(range(prev, hi)))
        prev = hi
    kt_ready[-1] = list(range(kt_ready[-1][0] if kt_ready[-1] else prev, NK))

    count_r = count.rearrange("p bt v f -> p bt (v f)")

    def transposes(kts):
        for kt in kts:
            for bt in range(NBT):
                p_t = psum.tile([128, 128], BF16)
                nc.tensor.transpose(p_t, count_r[:, bt, ts(kt, 128)], ident)
                nc.scalar.copy(out=ct_all[:, kt, bt, :], in_=p_t)

    pending = []
    for g in range(NG):
        ohG = ohpool.tile([128, VG, NBT, 4, QTR * F], BF16)
        compares(g, ohG)
        pending.append((g, ohG))
        if len(pending) > 1:
            gg, oo = pending.pop(0)
            tail(gg, oo)
            transposes(kt_ready[gg])
    while pending:
        gg, oo = pending.pop(0)
        tail(gg, oo)
        transposes(kt_ready[gg])

    # ---- main matmul: y_t [66, bt, 128] = T'^T @ count_t (+ dense/bias) ----
    y_t = psum1.tile([66, NBT, 128], FP32)
    for kt in range(NK):
        nc.tensor.matmul(
            y_t,
            tabT[:, kt, :],
            ct_all[:, kt, :, :],
            start=(kt == 0),
            stop=False,
        )
    nc.tensor.matmul(y_t, wdense, densebias_t, start=False, stop=True)

    # ---- epilogue: z = 0.5|s|^2 + y64 + y65; out = sigmoid(z) ----
    for bt in range(NBT):
        s2_sb = small.tile([64, 128], FP32)
        nc.scalar.activation(out=s2_sb, in_=y_t[0:64, bt, :], func=AF.Square)
        lin_sq = small.tile([2, 128], FP32)
        nc.scalar.copy(out=lin_sq, in_=y_t[64:66, bt, :])
        z_ps = psum.tile([128, 1], FP32, tag="pscratch")
        nc.tensor.matmul(z_ps, s2_sb, half_col, start=True, stop=False)
        nc.tensor.matmul(z_ps, lin_sq, ones2, start=False, stop=True)
        out_sb = small.tile([128, 1], FP32)
        nc.scalar.activation(out=out_sb, in_=z_ps, func=AF.Sigmoid)
        nc.sync.dma_start(out=y_d.ap()[bt * 128 : (bt + 1) * 128, :], in_=out_sb)


def build():
    if "nc" in _BUILT:
        return _BUILT["nc"]
    nc = bacc.Bacc("TRN2", target_bir_lowering=False, debug=False)
    idx_d = nc.dram_tensor("idx", [BC, R], BF16, kind="ExternalInput")
    dense_d = nc.dram_tensor("densebias", [14, NBT, 128], BF16, kind="ExternalInput")
    tab_d = nc.dram_tensor("tab", [R + 14, 66], BF16, kind="ExternalInput")
    y_d = nc.dram_tensor("y", [BC, 1], FP32, kind="ExternalOutput")
    with tile.TileContext(nc) as tc:
        with ExitStack() as ctx:
            _emit(ctx, tc, idx_d, dense_d, tab_d, y_d)
    nc.compile()
    _BUILT["nc"] = nc
    return nc


def make_in_maps(sparse_feat, dense_feat, lin_emb, emb_tab, lin_W, lin_b):
    bf = ml_dtypes.bfloat16
    idx = np.asarray(sparse_feat)[:, _COLS].astype(bf)
    dense = np.asarray(dense_feat, dtype=np.float32)
    emb = np.asarray(emb_tab, dtype=np.float32)
    linemb = np.asarray(lin_emb, dtype=np.float32).reshape(F, C)
    linw = np.asarray(lin_W, dtype=np.float32).reshape(F + D)
    linb = float(np.asarray(lin_b, dtype=np.float32).reshape(()))

    # host-fused table T' rows r=(v,f): [emb | -0.5|row|^2 | lin_emb*lin_W],
    # then 14 extra rows holding the dense weights + bias in col 65
    emb_vf = np.ascontiguousarray(emb.transpose(1, 0, 2))          # [C, F, E]
    emb_bf = emb_vf.astype(bf).astype(np.float32)
    tab = np.zeros((R + 14, 66), dtype=np.float32)
    tab[:R, 0:E] = emb_bf.reshape(R, E)
    tab[:R, E] = -0.5 * (emb_bf * emb_bf).sum(axis=2).reshape(R)
    tab[:R, E + 1] = (linemb.T * linw[None, :F]).reshape(R)
    tab[R : R + 13, 65] = linw[F:]
    tab[R + 13, 65] = linb
    tab = tab.astype(bf)

    in_maps = []
    for i in range(NCORES):
        sl = slice(i * BC, (i + 1) * BC)
        dsl = dense[sl]                                            # [256, 13]
        db = np.ones((14, NBT, 128), dtype=np.float32)
        db[0:13] = dsl.T.reshape(13, NBT, 128)
        in_maps.append(
            {
                "idx": np.ascontiguousarray(idx[sl]),
                "densebias": db.astype(bf),
                "tab": tab,
            }
        )
    return in_maps


def kernel(sparse_feat, dense_feat, lin_emb, emb_tab, lin_W, lin_b):
    nc = build()
    in_maps = make_in_maps(sparse_feat, dense_feat, lin_emb, emb_tab, lin_W, lin_b)
    res = run_bass_kernel_spmd(nc, in_maps, list(range(NCORES)))
    return np.concatenate([r["y"] for r in res.results], axis=0)
